# revision 1
# baseline (speedup 1.0000x reference)
"""Cross-attention kernel for 8 Trainium2 NeuronCores (Bass/Tile, SPMD).

Reference computation (per batch b of 4):
    K_proj = K[b] @ Wk.T + bk            # [2048, 1024]
    V_proj = V[b] @ Wv.T + bv            # [2048, 1024]
    S      = Q[b] @ K_proj.T / 32        # [1024, 2048]
    P      = softmax(S, axis=-1)
    ctx    = P @ V_proj                  # [1024, 1024]
    out[b] = ctx @ Wo.T + bo             # [1024, 1024]

Sharding: 8 cores = 4 batches x 2 query-halves. Each core handles one
batch element and 512 of its 1024 queries; the K/V projections are
recomputed on both cores of a batch pair (no cross-core communication).

Everything on-chip is computed transposed where it helps the PE:
  kpT  = K_proj.T  [d1, Lk]   (lhsT for nothing, rhs for S)
  S    = Q_h @ K_proj.T       [q, k]    -> softmax along free dim
  pT   = P.T via PE transposes [k, q]
  ctxT = V_proj.T @ P.T       [d, q]
  outT = Wo @ ctxT            [e, q]    -> host transposes back

All matmuls run as float32r (tf32-like, full PE rate at N=512);
producers write f32r-rounded values as the walrus verifier requires.

This container's walrus accepts at most ONE sync-wait command per
instruction (TPB ops, DMA descriptors and the Tile end-of-context
Drain alike).  Two local legalizations deal with that:
  * PatchedTileContext splits the final drain into one drain per
    outstanding proc.
  * split_multi_waits() hoists extra waits onto same-engine NoOps.
"""

import numpy as np

import concourse.bass as bass
import concourse.mybir as mybir
import concourse.tile as tile
from concourse.bass_utils import run_bass_kernel_spmd
from concourse.masks import make_identity
from bass_rust import ScopedClock, VectorClock
from contextlib import ExitStack

F32 = mybir.dt.float32
F32R = mybir.dt.float32r
AX = mybir.AxisListType.X
EXP = mybir.ActivationFunctionType.Exp

B = 4
D1 = 1024
D2 = 1280
LK = 2048
LQ = 512          # queries per core
N_CORES = 8
SCALE = 1.0 / 32.0  # 1/sqrt(D1)

NT1 = D1 // 128   # 8  d1 tiles
NT2 = D2 // 128   # 10 d2 tiles
NTK = LK // 128   # 16 key tiles
NQ = LQ // 128    # 4  query tiles per core
NKB = LK // 512   # 4  key blocks of 512


class PatchedTileContext(tile.TileContext):
    """Split the end-of-context drain into single-wait drains."""

    def _drain_and_barrier(self, tick_clock, wait_clock):
        gc = tick_clock.global_clock
        n = len(gc)
        for i in range(n):
            t = gc[i]
            if t > 0:
                vec = [0] * n
                vec[i] = t
                d = self.nc.sync.drain()
                wait_clock.add_sem_waits(
                    d.ins, ScopedClock({None: VectorClock(vec)})
                )
        self.nc.all_engine_barrier()
        assert self.sems is not None
        popped = self.nc._tile_sem_poison_stack.pop()
        assert popped is self._sem_poison
        self.nc.clear_and_free_semaphores(list(self.sems.allocated().values()))
        self.nc.all_engine_barrier()


def split_multi_waits(nc, limit=1):
    """Hoist waits beyond `limit` onto same-engine NoOps placed directly
    before the offending instruction. Engine streams execute in order and
    Tile emits each stream in dependency-topological order, so this is
    semantics-preserving."""
    n_split = 0
    for fn in nc.m.functions:
        for blk in fn.blocks:
            il = blk.instructions
            newlist = []
            changed = False
            for inst in il:
                si = inst.sync_info
                ow = list(si.on_wait) if si is not None else []
                if len(ow) > limit:
                    for k, w in enumerate(ow[:-limit]):
                        nop = mybir.InstNoOp(
                            name=f"{inst.name}-ws{k}", ins=[], outs=[]
                        )
                        nop.engine = inst.engine
                        nop.sync_info = mybir.SyncInfo(on_wait=[w], on_update=[])
                        newlist.append(nop)
                        n_split += 1
                    inst.sync_info = mybir.SyncInfo(
                        on_wait=ow[-limit:], on_update=list(si.on_update)
                    )
                    changed = True
                newlist.append(inst)
            if changed:
                del il[:]
                il.extend(newlist)
    return n_split


def build_program(n_rounds=1):
    nc = bass.Bass()

    QT = nc.dram_tensor("QT", [D1, LQ], F32, kind="ExternalInput")
    KT = nc.dram_tensor("KT", [D2, LK], F32, kind="ExternalInput")
    VT = nc.dram_tensor("VT", [D2, LK], F32, kind="ExternalInput")
    WkT = nc.dram_tensor("WkT", [D2, D1], F32, kind="ExternalInput")
    WvT = nc.dram_tensor("WvT", [D2, D1], F32, kind="ExternalInput")
    WoT = nc.dram_tensor("WoT", [D1, D1], F32, kind="ExternalInput")
    bkbo = nc.dram_tensor("bkbo", [128, 2 * NT1], F32, kind="ExternalInput")
    bvB = nc.dram_tensor("bvB", [128, D1], F32, kind="ExternalInput")
    outT = nc.dram_tensor("outT", [D1, LQ], F32, kind="ExternalOutput")

    with PatchedTileContext(nc) as tc:
        es_stats = ExitStack()
        # Persistent small tiles. bkbo: one DMA for all bias columns
        # (bk tiles in cols 0..7, bo tiles in cols 8..15).
        stats = es_stats.enter_context(tc.tile_pool(name="stats", bufs=1))
        statv = es_stats.enter_context(tc.tile_pool(name="statv", bufs=8))
        ident = stats.tile([128, 128], F32)
        make_identity(nc, ident[:])
        bias_t = stats.tile([128, 2 * NT1], F32)
        nc.sync.dma_start(bias_t[:], bkbo[:])
        bvB_t = stats.tile([128, D1], F32)

        def emit_round(rnd):
            sfx = f"_{rnd}"
            es_pwv = ExitStack()     # wvT (prefetched during A/B)
            es_a = ExitStack()       # wkT + KT stream
            es_p1 = ExitStack()      # kpT + qT (right side)
            es_sm = ExitStack()      # esb (softmax buffer)
            es_pt = ExitStack()      # pT (right side)
            es_c = ExitStack()       # VT stream
            es_vp = ExitStack()      # vp (right side)
            es_tail = ExitStack()    # woT + ctxT + osb
            es_ppa = ExitStack()
            es_pps = ExitStack()
            es_ppt = ExitStack()
            es_ppcd = ExitStack()

            # ---- phase A: kpT = Wk @ K.T  [d1, Lk] ----------------------
            # DMA priority order: wkT f-slices + first KT block feed the
            # first matmuls; qT / wvT / bvB are demoted below them.
            p1 = es_p1.enter_context(tc.tile_pool(name="p1" + sfx, bufs=1, side="right"))
            kpT = p1.tile([128, NT1 * LK], F32)   # tile m at [:, m*LK:(m+1)*LK]
            qT = p1.tile([128, NT1 * LQ], F32)    # tile f at [:, f*LQ:(f+1)*LQ]

            pwv = es_pwv.enter_context(tc.tile_pool(name="pwv" + sfx, bufs=1))
            pa = es_a.enter_context(tc.tile_pool(name="pa" + sfx, bufs=1))
            pa_s = es_a.enter_context(tc.tile_pool(name="pa_s" + sfx, bufs=2))
            ppa = es_ppa.enter_context(tc.tile_pool(name="ppa" + sfx, bufs=4, space="PSUM"))

            wk_t = [pa.tile([128, D1], F32, tag=f"wk{f}", name=f"wk{f}" + sfx)
                    for f in range(NT2)]
            wv_t = []
            for n in range(NKB):
                ks = [pa_s.tile([128, 512], F32, tag=f"ks{f}", name=f"ks{f}" + sfx) for f in range(NT2)]
                for f in range(NT2):
                    if n == 0:
                        nc.sync.dma_start(
                            wk_t[f][:].bitcast(F32R),
                            WkT[f * 128 : (f + 1) * 128, :].bitcast(F32R),
                        )
                    nc.sync.dma_start(
                        ks[f][:].bitcast(F32R),
                        KT[f * 128 : (f + 1) * 128, n * 512 : (n + 1) * 512].bitcast(F32R),
                    )
                for m in range(NT1):
                    ps = ppa.tile([128, 512], F32, tag="ppa")
                    for f in range(NT2):
                        nc.tensor.matmul(
                            ps[:],
                            wk_t[f][:, m * 128 : (m + 1) * 128].bitcast(F32R),
                            ks[f][:].bitcast(F32R),
                            start=(f == 0),
                            stop=(f == NT2 - 1),
                        )
                    nc.vector.tensor_scalar_add(
                        kpT[:, m * LK + n * 512 : m * LK + (n + 1) * 512].bitcast(F32R),
                        ps[:],
                        bias_t[:, m : m + 1],
                    )
                if n == NKB - 1:
                    # demoted loads: needed only from phase B / C onward
                    for f in range(NT1):
                        nc.sync.dma_start(
                            qT[:, f * LQ : (f + 1) * LQ].bitcast(F32R),
                            QT[f * 128 : (f + 1) * 128, :].bitcast(F32R),
                        )
                    for f in range(NT2):
                        w = pwv.tile([128, D1], F32, tag=f"wv{f}", name=f"wv{f}" + sfx)
                        nc.sync.dma_start(
                            w[:].bitcast(F32R),
                            WvT[f * 128 : (f + 1) * 128, :].bitcast(F32R),
                        )
                        wv_t.append(w)
                    nc.sync.dma_start(bvB_t[:], bvB[:])
            es_a.close()

            # ---- phase B: S = qT.T @ kpT, softmax along k ---------------
            # VT stream pool opens early so its first blocks land during B;
            # zone-reuse deps on phase-A readers pace them safely.
            pc_s = es_c.enter_context(tc.tile_pool(name="pc_s" + sfx, bufs=2))
            vs_blocks = {}
            for n in range(2):
                vs = [pc_s.tile([128, 512], F32, tag=f"vs{f}", name=f"vs{f}_{n}" + sfx) for f in range(NT2)]
                for f in range(NT2):
                    nc.sync.dma_start(
                        vs[f][:].bitcast(F32R),
                        VT[f * 128 : (f + 1) * 128, n * 512 : (n + 1) * 512].bitcast(F32R),
                    )
                vs_blocks[n] = vs
            sm = es_sm.enter_context(tc.tile_pool(name="sm" + sfx, bufs=1))
            esb = sm.tile([128, NQ * LK], F32)    # tile m at [:, m*LK:(m+1)*LK]
            es_ppa.close()
            # S psum split into two 2-bank halves (pool 6 banks) so the
            # transpose psum pool (2 banks) coexists — transposes of row
            # tile m overlap S matmuls of m+1.
            pps = es_pps.enter_context(tc.tile_pool(name="pps" + sfx, bufs=3, space="PSUM"))
            ppt = es_ppt.enter_context(tc.tile_pool(name="ppt" + sfx, bufs=2, space="PSUM"))
            for m in range(NQ):
                ph = [pps.tile([128, 1024], F32, tag="pps", name=f"ps{m}h{h}" + sfx)
                      for h in range(2)]
                for n in range(NKB):
                    ps = ph[n // 2]
                    off = (n % 2) * 512
                    for f in range(NT1):
                        nc.tensor.matmul(
                            ps[:, off : off + 512],
                            qT[:, f * LQ + m * 128 : f * LQ + (m + 1) * 128].bitcast(F32R),
                            kpT[:, f * LK + n * 512 : f * LK + (n + 1) * 512].bitcast(F32R),
                            start=(f == 0),
                            stop=(f == NT1 - 1),
                        )
                mr = [statv.tile([128, 1], F32, tag=f"mr{h}", name=f"mr{m}h{h}" + sfx)
                      for h in range(2)]
                for h in range(2):
                    nc.vector.reduce_max(mr[h][:], ph[h][:], axis=AX)
                mraw = statv.tile([128, 1], F32, tag="mraw")
                nc.vector.tensor_max(mraw[:], mr[0][:], mr[1][:])
                mneg = statv.tile([128, 1], F32, tag="mneg")
                nc.scalar.mul(mneg[:], mraw[:], -SCALE)
                ls = [statv.tile([128, 1], F32, tag=f"ls{h}", name=f"ls{m}h{h}" + sfx)
                      for h in range(2)]
                for h in range(2):
                    nc.scalar.activation(
                        esb[:, m * LK + h * 1024 : m * LK + (h + 1) * 1024],
                        ph[h][:],
                        EXP,
                        bias=mneg[:],
                        scale=SCALE,
                        accum_out=ls[h][:],
                    )
                lsum = statv.tile([128, 1], F32, tag="lsum")
                nc.vector.tensor_add(lsum[:], ls[0][:], ls[1][:])
                rinv = statv.tile([128, 1], F32, tag="rinv")
                nc.vector.reciprocal(rinv[:], lsum[:])
                nc.vector.tensor_scalar_mul(
                    esb[:, m * LK : (m + 1) * LK],
                    esb[:, m * LK : (m + 1) * LK],
                    rinv[:],
                )
            es_p1.close()
            pt = es_pt.enter_context(tc.tile_pool(name="pt" + sfx, bufs=1, side="right"))
            pT = pt.tile([128, NTK * LQ], F32)    # tile kt at [:, kt*LQ:(kt+1)*LQ]
            for m in range(NQ):
                for kt in range(NTK):
                    tp = ppt.tile([128, 128], F32, tag="ppt")
                    nc.tensor.transpose(
                        tp[:], esb[:, m * LK + kt * 128 : m * LK + (kt + 1) * 128], ident[:]
                    )
                    nc.vector.tensor_copy(
                        pT[:, kt * LQ + m * 128 : kt * LQ + (m + 1) * 128].bitcast(F32R),
                        tp[:],
                    )
            es_sm.close()

            # ---- phase C: vp = V_proj  [Lk, d1] -------------------------
            vpp = es_vp.enter_context(tc.tile_pool(name="vpp" + sfx, bufs=1, side="right"))
            vp = vpp.tile([128, NTK * D1], F32)   # tile kt at [:, kt*D1:(kt+1)*D1]
            es_ppt.close()
            es_pps.close()
            ppc = es_ppcd.enter_context(tc.tile_pool(name="ppc" + sfx, bufs=4, space="PSUM"))
            ppd = es_ppcd.enter_context(tc.tile_pool(name="ppd" + sfx, bufs=4, space="PSUM"))
            for n in range(NKB):
                if n in vs_blocks:
                    vs = vs_blocks[n]
                else:
                    vs = [pc_s.tile([128, 512], F32, tag=f"vs{f}", name=f"vs{f}_{n}" + sfx) for f in range(NT2)]
                    for f in range(NT2):
                        nc.sync.dma_start(
                            vs[f][:].bitcast(F32R),
                            VT[f * 128 : (f + 1) * 128, n * 512 : (n + 1) * 512].bitcast(F32R),
                        )
                for j in range(4):
                    kt = n * 4 + j
                    for dh in range(2):
                        ps = ppc.tile([128, 512], F32, tag="ppc")
                        for f in range(NT2):
                            nc.tensor.matmul(
                                ps[:],
                                vs[f][:, j * 128 : (j + 1) * 128].bitcast(F32R),
                                wv_t[f][:, dh * 512 : (dh + 1) * 512].bitcast(F32R),
                                start=(f == 0),
                                stop=(f == NT2 - 1),
                            )
                        nc.vector.tensor_add(
                            vp[:, kt * D1 + dh * 512 : kt * D1 + (dh + 1) * 512].bitcast(F32R),
                            ps[:],
                            bvB_t[:, dh * 512 : (dh + 1) * 512],
                        )
            es_c.close()
            es_pwv.close()

            # ---- phase D: ctxT = V_proj.T @ P.T  [d, q] -----------------
            ptail = es_tail.enter_context(tc.tile_pool(name="ptail" + sfx, bufs=1))
            posb = es_tail.enter_context(tc.tile_pool(name="posb" + sfx, bufs=2))
            ctxT = ptail.tile([128, NT1 * LQ], F32)
            woT = ptail.tile([128, NT1 * D1], F32)
            for f in range(NT1):
                nc.sync.dma_start(
                    woT[:, f * D1 : (f + 1) * D1].bitcast(F32R),
                    WoT[f * 128 : (f + 1) * 128, :].bitcast(F32R),
                )
            for dt in range(NT1):
                ps = ppd.tile([128, LQ], F32, tag="ppd")
                for kt in range(NTK):
                    nc.tensor.matmul(
                        ps[:],
                        vp[:, kt * D1 + dt * 128 : kt * D1 + (dt + 1) * 128].bitcast(F32R),
                        pT[:, kt * LQ : (kt + 1) * LQ].bitcast(F32R),
                        start=(kt == 0),
                        stop=(kt == NTK - 1),
                    )
                nc.vector.tensor_copy(
                    ctxT[:, dt * LQ : (dt + 1) * LQ].bitcast(F32R), ps[:]
                )
            es_vp.close()
            es_pt.close()

            # ---- phase E: outT = Wo @ ctxT + bo  [e, q] -----------------
            for et in range(NT1):
                ps = ppd.tile([128, LQ], F32, tag="ppd")
                for dt in range(NT1):
                    nc.tensor.matmul(
                        ps[:],
                        woT[:, dt * D1 + et * 128 : dt * D1 + (et + 1) * 128].bitcast(F32R),
                        ctxT[:, dt * LQ : (dt + 1) * LQ].bitcast(F32R),
                        start=(dt == 0),
                        stop=(dt == NT1 - 1),
                    )
                ob = posb.tile([128, LQ], F32, tag="osb")
                nc.vector.tensor_scalar_add(ob[:], ps[:], bias_t[:, NT1 + et : NT1 + et + 1])
                nc.sync.dma_start(outT[et * 128 : (et + 1) * 128, :], ob[:])
            es_ppcd.close()
            es_tail.close()

        for rnd in range(n_rounds):
            emit_round(rnd)
        es_stats.close()

    split_multi_waits(nc)
    return nc


_PROGRAM = None


def _get_program():
    global _PROGRAM
    if _PROGRAM is None:
        _PROGRAM = build_program()
    return _PROGRAM


def build_in_maps(inputs):
    Q = np.asarray(inputs["Q"], dtype=np.float32)
    K = np.asarray(inputs["K"], dtype=np.float32)
    V = np.asarray(inputs["V"], dtype=np.float32)
    Wk = np.asarray(inputs["Wk"], dtype=np.float32)
    Wv = np.asarray(inputs["Wv"], dtype=np.float32)
    Wo = np.asarray(inputs["Wo"], dtype=np.float32)
    bk = np.asarray(inputs["bk"], dtype=np.float32)
    bv = np.asarray(inputs["bv"], dtype=np.float32)
    bo = np.asarray(inputs["bo"], dtype=np.float32)

    WkT_h = np.ascontiguousarray(Wk.T)            # [D2, D1]
    WvT_h = np.ascontiguousarray(Wv.T)
    WoT_h = np.ascontiguousarray(Wo.T)            # [D1, D1]
    bkbo_h = np.concatenate(
        [bk.reshape(NT1, 128).T, bo.reshape(NT1, 128).T], axis=1
    ).astype(np.float32).copy()
    bvB_h = np.ascontiguousarray(np.broadcast_to(bv, (128, D1)))
    KT_h = [np.ascontiguousarray(K[b].T) for b in range(B)]   # [D2, LK]
    VT_h = [np.ascontiguousarray(V[b].T) for b in range(B)]

    in_maps = []
    for c in range(N_CORES):
        b, h = divmod(c, 2)
        in_maps.append(
            {
                "QT": np.ascontiguousarray(Q[b, h * LQ : (h + 1) * LQ, :].T),
                "KT": KT_h[b],
                "VT": VT_h[b],
                "WkT": WkT_h,
                "WvT": WvT_h,
                "WoT": WoT_h,
                "bkbo": bkbo_h,
                "bvB": bvB_h,
            }
        )
    return in_maps


def assemble_output(results):
    out = np.empty((B, 2 * LQ, D1), dtype=np.float32)
    for c in range(N_CORES):
        b, h = divmod(c, 2)
        out[b, h * LQ : (h + 1) * LQ, :] = results[c]["outT"].T
    return out


def kernel(Q, K, V, Wk, bk, Wv, bv, Wo, bo):
    inputs = dict(Q=Q, K=K, V=V, Wk=Wk, bk=bk, Wv=Wv, bv=bv, Wo=Wo, bo=bo)
    nc = _get_program()
    in_maps = build_in_maps(inputs)
    res = run_bass_kernel_spmd(nc, in_maps, list(range(N_CORES)))
    return assemble_output(res.results)

